# revision 23
# baseline (speedup 1.0000x reference)
"""Trainium2 Bass kernel for nn_DeltaResidualExpanded.

Computes, per (b, t) position:
    k    = l2normalize(sublayer_output) / sqrt(D)
    beta = 2*sigmoid(RMSNorm(x_in) @ gate_w.T + gate_b)
    v    = x_in @ Wv.T
    out  = X + beta * k (outer) (v - k.X)

Pure data-parallel over B*T across 8 NeuronCores; each core streams its
1024 positions as 8 tiles of 128 through SBUF.  All big tensors are fp16
(host casts): the update term is a ~1e-3 relative perturbation of X, so
fp16 I/O costs ~5e-4 relative error vs the 2e-2 gate and halves HBM
traffic to ~36 MB/core (~105us at 358 GB/s/core).

X is laid out [pos, DV, D] (d innermost; host pre/post-transposes) so
every elementwise op is unit-stride.  Engine budget per 128-pos tile
(DMA ~12.5us/tile is the roofline):
  - ACT uses ONLY the natural_log_exp_and_others table set (ln, exp,
    square, copy all co-reside) so there is ONE ACT_TABLE_LOAD for the
    whole kernel (the old Sqrt+Sigmoid mix reloaded tables twice per
    tile, ~21us/core): 1/sqrt(x) = exp(-0.5*ln(x)); sigmoid via exp +
    DVE reciprocal.  ACT (~110us/core): all 8 kTX copy-accum reduces +
    the ssq Square-accum + the 3 tiny ln/exp ops + the gram PSUM copy.
  - DVE (~114us/core, the pacer): tmp = X*sub multiply (2x tt) feeding
    the ACT reduces, 8 delta tensor_scalar (per-partition corr2
    scalar), the X+=delta add tt in two j-halves (each half's store
    starts while the other adds), xt/vg/vgt PSUM copies, the gram-diag
    masked accum for msq, small per-position algebra.
  - PE: xin^T 128x128 transposes, matmul against [Wv.T | gate_w], and
    an xin gram matmul whose diagonal (via masked DVE stt from an ACT
    SBUF copy, never straight from PSUM) is mean(xin^2).
  - Software pipelining: each tile's reduce-DEPENDENT tail (ktx ladder,
    deltas, adds, stores — stage2) is emitted AFTER the next tile's
    stage1, so the in-order DVE queue never stalls on the ACT reduce
    train with ready work stuck behind it.  This cut both the mid-run
    stalls and the run-to-run variance (~154-160us vs 154-189us).

Measured dead ends (traces in the session log): scalar_tensor_tensor
runs at 1x (a fused update is SLOWER than delta+add), GPSIMD tt is
~3.9ns/elem and Pool rejects TensorScalarPtr, PSUM-sourced stt accums
cost ~0.8us, tensor_tensor_reduce doesn't codegen on this walrus, ACT
activations cannot write PSUM here (tensor3d_valid), a d-major layout
putting kTX+outer-product on PE loses to the PSUM-bank 512-col matmul
split + extra DMA, and hand-staged software pipelining loses to the
Tile scheduler's own cross-tile overlap.

The walrus build in this container accepts at most ONE on_wait condition
per instruction, so the Tile-scheduled BIR is post-processed to hoist
extra waits into standalone EventSemaphore instructions (legalize_bir).
"""
import sys
import math

sys.path.insert(0, "/opt/trn_rl_repo")

import numpy as np

B, T, D, DV = 4, 2048, 1024, 8
N_CORES = 8
BT = B * T
CORE_BT = BT // N_CORES          # 1024 positions per core
P = 128                          # partitions per tile
NT = CORE_BT // P                # 8 tiles per core
NC_D = D // P                    # 8 d-chunks of 128
EPS_K = 1e-6
EPS_NORM = 1e-6
W_COLS = DV + 2                  # Wv rows, gate row, zero pad
NSLOT = DV + 2                   # xx slots: 8 X rows, sub, xin

_NC_CACHE: dict = {}


def legalize_bir_dict(d):
    """Split multi-wait instructions (this walrus accepts one on_wait per
    instruction): hoist extras into standalone EventSemaphore instrs."""
    n = 0
    for fn in d.get("functions", []):
        for blk in fn.get("blocks", []):
            insts = blk.get("instructions")
            if not insts:
                continue
            out = []
            for inst in insts:
                si = inst.get("sync_info")
                waits = (si or {}).get("on_wait") or []
                if len(waits) > 1:
                    for w in waits[:-1]:
                        n += 1
                        out.append({
                            "debug": inst.get("debug", 0),
                            "engine": inst["engine"],
                            "ins": [],
                            "name": f"legwait-{n}",
                            "opcode": "EventSemaphore",
                            "outs": [],
                            "sync_info": {"on_update": [], "on_wait": [w]},
                        })
                    si["on_wait"] = waits[-1:]
                out.append(inst)
            blk["instructions"] = out
    return d


def _build(gate_b_val: float, reps: int = 1, opts: dict | None = None):
    opts = dict(opts or {})
    act_reduce = opts.get("act_reduce", 8)   # kTX reduces on ACT (rest DVE)
    msq_pe = opts.get("msq_pe", True)        # xin^2 norm via PE gram
    ssq_dve = opts.get("ssq_dve", False)     # sub^2 norm on DVE (not ACT)
    gps_add = opts.get("gps_add", 0)         # final-add j slices on GPSIMD
    split_store = opts.get("split_store", 1)
    xbufs = opts.get("xbufs", 6)

    import orjson
    import concourse.bass as bass
    import concourse.tile as tile
    from concourse import mybir, masks
    from concourse.bass import ts
    from contextlib import ExitStack

    f32 = mybir.dt.float32
    f16 = mybir.dt.float16
    AF = mybir.ActivationFunctionType
    OP = mybir.AluOpType

    dve_reduce = DV - act_reduce
    assert dve_reduce >= 0

    nc = bass.Bass()
    X = nc.dram_tensor("X", [CORE_BT, DV, D], f16, kind="ExternalInput")
    # SX = [sublayer_output | x_in] fused along the feature axis
    SX = nc.dram_tensor("SX", [CORE_BT, 2 * D], f16, kind="ExternalInput")
    # [D, W_COLS]: cols 0..7 = Wv.T, col 8 = gate_norm_w*gate_w, col 9 = 0
    WT = nc.dram_tensor("WT", [D, W_COLS], f16, kind="ExternalInput")
    OUT = nc.dram_tensor("OUT", [CORE_BT, DV, D], f16, kind="ExternalOutput")

    with tile.TileContext(nc) as tc, ExitStack() as ctx:
        consts = ctx.enter_context(tc.tile_pool(name="consts", bufs=1))
        xap = ctx.enter_context(tc.tile_pool(name="xap", bufs=xbufs))
        delp = ctx.enter_context(tc.tile_pool(name="delp", bufs=1))
        tmpp = ctx.enter_context(tc.tile_pool(name="tmpp", bufs=2))
        xtp = ctx.enter_context(tc.tile_pool(name="xtp", bufs=2))
        scrp = ctx.enter_context(tc.tile_pool(name="scrp", bufs=2))
        small = ctx.enter_context(tc.tile_pool(name="small", bufs=3))
        tpsum = ctx.enter_context(tc.tile_pool(name="tpsum", bufs=2,
                                               space="PSUM"))
        vpsum = ctx.enter_context(tc.tile_pool(name="vpsum", bufs=2,
                                               space="PSUM"))
        wpsum = ctx.enter_context(tc.tile_pool(name="wpsum", bufs=2,
                                               space="PSUM"))
        if msq_pe:
            gpsum = ctx.enter_context(tc.tile_pool(name="gpsum", bufs=1,
                                                   space="PSUM"))

        eps_sb = consts.tile([P, 1], f32)
        nc.vector.memset(eps_sb, EPS_NORM)
        ngb_sb = consts.tile([P, 1], f32)
        nc.vector.memset(ngb_sb, -gate_b_val)
        ident = consts.tile([P, P], f16)
        masks.make_identity(nc, ident[:])
        # WT load as [128 d-in-chunk, chunk, col]
        wt_sb = consts.tile([P, NC_D, W_COLS], f16)
        nc.gpsimd.dma_start(
            out=wt_sb, in_=WT[:].rearrange("(c p) m -> p c m", p=P))

        def stage2(xx, sub_t, raw, sinv, bs, vgt, rows):
            """Reduce-dependent tail of a tile (ktx ladder, delta, += X,
            store), emitted AFTER the next tile's stage1: by then the ACT
            reduce train feeding `raw` has finished, so the in-order DVE
            queue never stalls on it with ready work stuck behind."""
            ktx = small.tile([P, DV], f32, tag="ktx")
            nc.vector.tensor_scalar(out=ktx, in0=raw, scalar1=sinv,
                                    scalar2=1.0 / math.sqrt(D),
                                    op0=OP.mult, op1=OP.mult)
            corr = small.tile([P, DV], f32, tag="corr")
            nc.vector.tensor_tensor(out=corr, in0=vgt[:, 0:DV], in1=ktx,
                                    op=OP.subtract)
            corr2 = small.tile([P, DV], f32, tag="corr2")
            nc.vector.tensor_scalar_mul(out=corr2, in0=corr, scalar1=bs)
            delta = delp.tile([P, DV, D], f16, tag="del")
            for j in range(DV):
                nc.vector.tensor_scalar(out=delta[:, j, :], in0=sub_t,
                                        scalar1=corr2[:, j:j + 1],
                                        scalar2=None, op0=OP.mult)
            # add + store in j-halves: the first store starts while the
            # second half is still adding (shorter drain tail)
            h = DV // 2
            for s in range(2):
                sl = slice(s * h, (s + 1) * h)
                nc.vector.tensor_tensor(out=xx[:, sl, :], in0=xx[:, sl, :],
                                        in1=delta[:, sl, :], op=OP.add)
                nc.scalar.dma_start(out=OUT[rows, sl, :], in_=xx[:, sl, :])

        pend = None
        for t in range(NT * reps):
            t = t % NT
            rows = slice(t * P, (t + 1) * P)

            # xx = [X rows 0..7 | sub | xin], one contiguous SBUF tile.
            # SX issued first so the PE gate/v path starts ASAP.
            xx = xap.tile([P, NSLOT, D], f16)
            nc.sync.dma_start(
                out=xx[:, DV:NSLOT, :],
                in_=SX[rows].rearrange("p (s d) -> p s d", s=2))
            nc.sync.dma_start(out=xx[:, 0:DV, :], in_=X[rows])
            sub_t = xx[:, DV, :]
            xin_t = xx[:, DV + 1, :]

            # ---- v & gate dot via PE: xin^T chunks, matmul with WT
            xt_ps = tpsum.tile([P, NC_D, P], f16, tag="tp")
            for c in range(NC_D):
                nc.tensor.transpose(xt_ps[:, c, :], xin_t[:, ts(c, P)],
                                    ident[:])
            xt_sb = xtp.tile([P, NC_D, P], f16)
            nc.vector.tensor_scalar(out=xt_sb, in0=xt_ps[:],
                                    scalar1=1.0, scalar2=None,
                                    op0=OP.mult)
            vg_ps = vpsum.tile([W_COLS, P], f32, tag="vg")
            for c in range(NC_D):
                nc.tensor.matmul(vg_ps[:, :], wt_sb[:, c, :],
                                 xt_sb[:, c, :],
                                 start=(c == 0), stop=(c == NC_D - 1))
            vg_sb = small.tile([W_COLS, P], f16, tag="vgsb")
            nc.vector.tensor_scalar(out=vg_sb, in0=vg_ps[:],
                                    scalar1=1.0, scalar2=None, op0=OP.mult)
            vgt_ps = wpsum.tile([P, W_COLS], f16, tag="vgt")
            nc.tensor.transpose(vgt_ps[:], vg_sb[:],
                                ident[:W_COLS, :W_COLS])
            vgt = small.tile([P, W_COLS], f32, tag="vgt_sb")
            nc.vector.tensor_scalar(out=vgt, in0=vgt_ps[:],
                                    scalar1=1.0, scalar2=None, op0=OP.mult)

            # ---- norms: nrm2 = [||sub||^2, mean(xin^2)]
            nrm2 = small.tile([P, 2], f32)
            if ssq_dve:
                sq0 = scrp.tile([P, D], f16, tag="sqd")
                nc.vector.scalar_tensor_tensor(
                    out=sq0, in0=sub_t, scalar=1.0, in1=sub_t,
                    op0=OP.mult, op1=OP.mult, accum_out=nrm2[:, 0:1])
            else:
                scr_n = scrp.tile([P, D], f16, tag="scrn")
                nc.scalar.activation(out=scr_n, in_=sub_t, func=AF.Square,
                                     accum_out=nrm2[:, 0:1])
            if msq_pe:
                gram_ps = gpsum.tile([P, P], f32, tag="gram")
                for c in range(NC_D):
                    nc.tensor.matmul(gram_ps[:, :], xt_sb[:, c, :],
                                     xt_sb[:, c, :],
                                     start=(c == 0), stop=(c == NC_D - 1))
                gram_sb = small.tile([P, P], f16, tag="gramsb")
                nc.scalar.copy(out=gram_sb, in_=gram_ps[:])
                gsc = small.tile([P, P], f16, tag="gsc")
                nc.vector.scalar_tensor_tensor(
                    out=gsc, in0=gram_sb[:], scalar=1.0 / D, in1=ident[:],
                    op0=OP.mult, op1=OP.mult, accum_out=nrm2[:, 1:2])
            else:
                scr_m = scrp.tile([P, D], f16, tag="scrm")
                nc.scalar.activation(out=scr_m, in_=xin_t, func=AF.Square,
                                     scale=1.0 / math.sqrt(D),
                                     accum_out=nrm2[:, 1:2])
            # inrm = [1/||sub||, 1/rmsden] = exp(-0.5*ln(nrm2 + eps))
            # (single ACT table set: ln+exp+square+copy co-reside)
            lnv = small.tile([P, 2], f32)
            nc.scalar.activation(out=lnv, in_=nrm2, func=AF.Ln,
                                 bias=eps_sb)
            inrm = small.tile([P, 2], f32)
            nc.scalar.activation(out=inrm, in_=lnv, func=AF.Exp,
                                 scale=-0.5)
            sinv = inrm[:, 0:1]
            # minrm = -inrm (for the sigmoid exp's scale)
            minrm = small.tile([P, 2], f32)
            nc.vector.tensor_scalar(out=minrm, in0=inrm, scalar1=-1.0,
                                    scalar2=None, op0=OP.mult)

            # ---- sig = sigmoid(g*rms + gate_b) = 1/(1 + exp(-(g*rms+gb)))
            ee = small.tile([P, 1], f32)
            nc.scalar.activation(out=ee, in_=vgt[:, DV:DV + 1],
                                 func=AF.Exp, scale=minrm[:, 1:2],
                                 bias=ngb_sb)
            den = small.tile([P, 1], f32)
            nc.vector.tensor_scalar(out=den, in0=ee, scalar1=1.0,
                                    scalar2=None, op0=OP.add)
            sig = small.tile([P, 1], f32)
            nc.vector.reciprocal(out=sig, in_=den)
            # bs = 2*sig*sinv/sqrt(D)
            bs = small.tile([P, 1], f32)
            nc.vector.tensor_scalar(out=bs, in0=sig, scalar1=sinv,
                                    scalar2=2.0 / math.sqrt(D),
                                    op0=OP.mult, op1=OP.mult)

            # ---- raw[:, j] = sum_d X[:, j, :]*sub
            # ACT j's: one partial tmp = X*sub multiply (2x tt), then
            # copy-accum per j.  GPSIMD/DVE j's: fused stt accum.
            raw = small.tile([P, DV], f32)
            if act_reduce > 0:
                sub_b = bass.AP(tensor=sub_t.tensor, offset=sub_t.offset,
                                ap=[sub_t.ap[0], [0, act_reduce],
                                    sub_t.ap[1]])
                tmp = tmpp.tile([P, act_reduce, D], f16, tag="tmp")
                nc.vector.tensor_tensor(out=tmp, in0=xx[:, 0:act_reduce, :],
                                        in1=sub_b, op=OP.mult)
                scr = scrp.tile([P, D], f16, tag="scr")
                for j in range(act_reduce):
                    nc.scalar.activation(out=scr, in_=tmp[:, j, :],
                                         func=AF.Copy,
                                         accum_out=raw[:, j:j + 1])
            for j in range(act_reduce, DV):
                kx = scrp.tile([P, D], f16, tag=f"kx{j}")
                nc.vector.scalar_tensor_tensor(
                    out=kx, in0=xx[:, j, :], scalar=1.0, in1=sub_t,
                    op0=OP.mult, op1=OP.mult, accum_out=raw[:, j:j + 1])

            # previous tile's reduce-dependent tail (see stage2)
            if pend is not None:
                stage2(*pend)
            pend = (xx, sub_t, raw, sinv, bs, vgt, rows)
        if pend is not None:
            stage2(*pend)

    legal = orjson.dumps(legalize_bir_dict(nc.to_json()))
    nc.to_json_bytes = lambda: legal  # consumed by bass2jax custom-call
    return nc


def get_nc(gate_b_val: float, reps: int = 1, opts: dict | None = None):
    key = (float(gate_b_val), reps, tuple(sorted((opts or {}).items())))
    if key not in _NC_CACHE:
        _NC_CACHE[key] = _build(gate_b_val, reps, opts)
    return _NC_CACHE[key]


def make_in_maps(X, sublayer_output, x_in, gate_norm_w, gate_w, Wv):
    # [BT, DV, D]: d innermost so device-side per-j slices are unit-stride
    Xf = np.ascontiguousarray(
        np.asarray(X, dtype=np.float32).reshape(BT, D, DV)
        .astype(np.float16).transpose(0, 2, 1))
    SXf = np.concatenate(
        [np.asarray(sublayer_output, dtype=np.float32).reshape(BT, D),
         np.asarray(x_in, dtype=np.float32).reshape(BT, D)],
        axis=1).astype(np.float16)
    gw = (np.asarray(gate_w, dtype=np.float32).reshape(D)
          * np.asarray(gate_norm_w, dtype=np.float32).reshape(D))
    WTv = np.zeros((D, W_COLS), dtype=np.float32)
    WTv[:, :DV] = np.asarray(Wv, dtype=np.float32).T
    WTv[:, DV] = gw
    WTv = WTv.astype(np.float16)
    in_maps = []
    for c in range(N_CORES):
        sl = slice(c * CORE_BT, (c + 1) * CORE_BT)
        in_maps.append({"X": Xf[sl], "SX": SXf[sl], "WT": WTv})
    return in_maps


def kernel(X, sublayer_output, x_in, gate_norm_w, gate_w, gate_b, Wv):
    from concourse.bass_utils import run_bass_kernel_spmd

    gate_b_val = float(np.asarray(gate_b).reshape(-1)[0])
    nc = get_nc(gate_b_val)
    in_maps = make_in_maps(X, sublayer_output, x_in, gate_norm_w, gate_w, Wv)
    res = run_bass_kernel_spmd(nc, in_maps, list(range(N_CORES)))
    out = np.concatenate([res.results[c]["OUT"] for c in range(N_CORES)],
                         axis=0)                     # [BT, DV, D]
    return np.ascontiguousarray(
        out.reshape(B, T, DV, D).transpose(0, 1, 3, 2)).astype(np.float32)
